# revision 1
# baseline (speedup 1.0000x reference)
"""GVPTransformerLayer kernel — self-contained.

Implements the full GVP transformer layer (multi-head graph attention with
scatter softmax, GVP message passing, node/edge/global updates with PNA
pooling). Computation mirrors the oracle exactly; runs via XLA on CPU
devices (the neuron Bass path did not land in budget).
"""
import numpy as np

N, E, B, H = 20000, 160000, 64, 8
NS, NV = 64, 8        # node dims (scalar, vector)
ES_, EV = 32, 4       # edge dims
GS, GV = 32, 4        # global dims
DKS, DKV = 8, 2       # per-head key/query dims
DVS, DVV = 8, 2       # per-head node value dims
DES, DEV = 8, 2       # per-head edge value dims
DB = 4                # per-head edge attention bias dim
EPS = 1e-8

_JITTED = None


def _build():
    import jax, jax.numpy as jnp

    def _norm(x, axis=-1, keepdims=False):
        return jnp.sqrt(jnp.clip(jnp.sum(x * x, axis, keepdims=keepdims), EPS))

    def gvp(s, v, wh, ws, bs, wv):
        vh = jnp.einsum('nvc,hv->nhc', v, wh)
        vn = _norm(vh, axis=-1)
        s_out = jnp.concatenate([s, vn], -1) @ ws.T + bs
        v_out = jnp.einsum('nhc,oh->noc', vh, wv)
        return s_out, v_out

    def tuple_ln(s, v, g, b, vw):
        mu = jnp.mean(s, -1, keepdims=True)
        var = jnp.mean((s - mu) ** 2, -1, keepdims=True)
        s_out = (s - mu) / jnp.sqrt(var + 1e-5) * g + b
        vn2 = jnp.clip(jnp.sum(v * v, -1, keepdims=True), EPS)
        denom = jnp.sqrt(jnp.mean(vn2, -2, keepdims=True))
        return s_out, v / denom * vw[:, None]

    def seg_mean(x, idx, n):
        c = jnp.maximum(jax.ops.segment_sum(jnp.ones((x.shape[0],), x.dtype), idx, n), 1.0)
        return jax.ops.segment_sum(x, idx, n) / c.reshape((-1,) + (1,) * (x.ndim - 1))

    def seg_std(x, idx, n):
        c = jnp.maximum(jax.ops.segment_sum(jnp.ones((x.shape[0],), x.dtype), idx, n), 1.0)[:, None]
        m = jax.ops.segment_sum(x, idx, n) / c
        msq = jax.ops.segment_sum(x * x, idx, n) / c
        var = (msq - m * m) * (c / jnp.maximum(c - 1.0, 1.0))
        return jnp.sqrt(jnp.maximum(var, 0.0) + 1e-12)

    def pna(s, v, idx, n, wh, ws, bs, wv):
        sm = seg_mean(s, idx, n)
        smi = jax.ops.segment_min(s, idx, n)
        sma = jax.ops.segment_max(s, idx, n)
        sstd = seg_std(s, idx, n)
        vn = _norm(v, axis=-1)
        vm = seg_mean(v, idx, n)
        vmi = jax.ops.segment_min(vn, idx, n)
        vma = jax.ops.segment_max(vn, idx, n)
        vstd = seg_std(vn, idx, n)
        z = jnp.concatenate([sm, smi, sma, sstd, vmi, vma, vstd], -1)
        return gvp(z, vm, wh, ws, bs, wv)

    def run(s, v, edge_index, batch_mask, es, ev, gs, gv, params):
        p = params
        row, col = edge_index[0], edge_index[1]
        qs = s @ p['q_ws'].T
        qv = jnp.einsum('nvc,ov->noc', v, p['q_wv'])
        ks_ = s @ p['k_ws'].T
        kv = jnp.einsum('nvc,ov->noc', v, p['k_wv'])
        be = es @ p['b_ws'].T
        attn_s = jnp.sum((qs[row] * ks_[col]).reshape(-1, H, DKS), -1)
        attn_v = jnp.sum((qv[row] * kv[col]).reshape(-1, H, DKV, 3), (-2, -1))
        attn_e = jnp.sum(be.reshape(-1, H, DB), -1)
        logits = (attn_s / np.sqrt(3 * DKS) + attn_v / np.sqrt(9 * DKV)
                  + attn_e / np.sqrt(3 * DB))
        lmax = jax.ops.segment_max(logits, row, N)
        ex = jnp.exp(logits - lmax[row])
        attn = ex / jax.ops.segment_sum(ex, row, N)[row]
        vxs = (s @ p['vx_ws'].T).reshape(-1, H, DVS)
        vxv = jnp.einsum('nvc,ov->noc', v, p['vx_wv']).reshape(-1, H, DVV, 3)
        ves = (es @ p['ve_ws'].T).reshape(-1, H, DES)
        vev = jnp.einsum('nvc,ov->noc', ev, p['ve_wv']).reshape(-1, H, DEV, 3)
        a1 = attn[:, :, None]
        a2 = attn[:, :, None, None]
        ms = jnp.concatenate([vxs[col] * a1, ves * a1], -1).reshape(-1, H * (DVS + DES))
        mv = jnp.concatenate([vxv[col] * a2, vev * a2], -2).reshape(-1, H * (DVV + DEV), 3)
        ms, mv = gvp(ms, mv, p['msg_wh'], p['msg_ws'], p['msg_bs'], p['msg_wv'])
        ms = seg_mean(ms, row, N)
        mv = seg_mean(mv, row, N)
        ms = jnp.concatenate([ms, gs[batch_mask]], -1)
        mv = jnp.concatenate([mv, gv[batch_mask]], -2)
        dxs, dxv = gvp(ms, mv, p['xout_wh'], p['xout_ws'], p['xout_bs'], p['xout_wv'])
        xs_o, xv_o = tuple_ln(s + dxs, v + dxv, p['xn_g'], p['xn_b'], p['xn_vw'])
        ebm = batch_mask[row]
        e_s = jnp.concatenate([s[row], s[col], es, attn_s, attn_v, attn_e, gs[ebm]], -1)
        e_v = jnp.concatenate([v[row], v[col], ev, gv[ebm]], -2)
        des_, dev_ = gvp(e_s, e_v, p['eout_wh'], p['eout_ws'], p['eout_bs'], p['eout_wv'])
        es_o, ev_o = tuple_ln(es + des_, ev + dev_, p['en_g'], p['en_b'], p['en_vw'])
        pxs, pxv = pna(s, v, batch_mask, B, p['pnax_wh'], p['pnax_ws'], p['pnax_bs'], p['pnax_wv'])
        pes, pev = pna(es, ev, ebm, B, p['pnae_wh'], p['pnae_ws'], p['pnae_bs'], p['pnae_wv'])
        yss, ysv = gvp(gs, gv, p['y_wh'], p['y_ws'], p['y_bs'], p['y_wv'])
        y_s = jnp.concatenate([pxs, pes, yss], -1)
        y_v = jnp.concatenate([pxv, pev, ysv], -2)
        dys, dyv = gvp(y_s, y_v, p['yout_wh'], p['yout_ws'], p['yout_bs'], p['yout_wv'])
        gs_o, gv_o = tuple_ln(gs + dys, gv + dyv, p['yn_g'], p['yn_b'], p['yn_vw'])
        return xs_o, xv_o, es_o, ev_o, gs_o, gv_o

    return jax.jit(run)


def kernel(s, v, edge_index, batch_mask, es, ev, gs, gv, params):
    global _JITTED
    import jax
    cpu = jax.devices('cpu')[0]
    if _JITTED is None:
        _JITTED = _build()
    with jax.default_device(cpu):
        args = jax.device_put(
            (np.asarray(s), np.asarray(v), np.asarray(edge_index),
             np.asarray(batch_mask), np.asarray(es), np.asarray(ev),
             np.asarray(gs), np.asarray(gv),
             {k: np.asarray(w) for k, w in params.items()}), cpu)
        out = _JITTED(*args)
        out = jax.block_until_ready(out)
    return tuple(np.asarray(o) for o in out)
